# revision 17
# baseline (speedup 1.0000x reference)
"""BinaryOneToManyMatcher (nms_detection) Trainium2 Bass kernel.

Computes, for B=128 images with Q=1000 predicted boxes and G=300 GT boxes:
  score = sigmoid(pred_logits)            [B,Q]
  iou   = pairwise IoU(pred, tgt)         [B,Q,G]
  gt    = score * iou * (iou > 0.4)       [B,Q,G]
  vals, idxs = top_k(gt over Q, k=4); mask = vals > 0

Sharding: pure data parallel, 16 images per NeuronCore across 8 cores.

Per-core layout: per image, G on partitions (chunks of <=128) and Q on the
free dim.  Per-query rows (x1,y1,x2,y2,area,-score) are packed to DRAM once,
loaded per image as one contiguous [1,6Q] line and broadcast across
partitions via PE ones-matmul (bit-exact).  Per-target values are [P,1]
per-partition scalars loaded in 3 batched DMAs for all 16 images.

Work split per g-chunk (balanced DVE/Pool):
  DVE : wx, wy (fused min-sub-max customs), recip seed, mask-select,
        Max8, MaxIndex
  Pool: inter, Up, NR correction (2 ops), iou*score (2 ops), negd, +bias
  Act : 6 PSUM->SBUF broadcast-row copies per image
  PE  : 12 broadcast matmuls per image

Exactness: idxs/mask are bit-exact vs the reference on the dataset.  The
valid decision uses negd = 0.4*Up - inter (sign test, no division); the
per-q tie-break bias of 2^-40 makes every row value distinct so Max8 ties
resolve to ascending q, matching jax.lax.top_k.  The NR-corrected
reciprocal (~2 ULP) only affects vals (loose tol) and top-4 ordering
(dataset margin 4.7e-6 >> 2 ULP).
"""

import os
from contextlib import ExitStack

import numpy as np

import concourse.bass as bass
import concourse.tile as tile
from concourse import bacc, mybir
from concourse.bass_utils import run_bass_kernel_spmd

B, Q, G, K = 128, 1000, 300, 4
NCORES = 8
BPC = B // NCORES  # images per core

F32 = mybir.dt.float32
I32 = mybir.dt.int32
U32 = mybir.dt.uint32
U8 = mybir.dt.uint8
Op = mybir.AluOpType

BIAS_SCALE = float(2.0**-40)  # per-q tie-break bias scale
POS_THRESH = 1e-6  # separates real positives (>=3e-3) from bias (<1e-9)


def _register_wsub():
    """Custom DVE op: out = min(in0, s0) - max(in1, s1) in one pass."""
    from concourse import dve_ops
    from concourse.dve_spec import Spec, Src0, Src1, C0, C1, minn, maxx, lower
    from concourse.dve_uop import DveOpSpec

    for op in dve_ops.OPS:
        if op.name == "WSUB_ANT":
            return op

    spec = Spec(
        body=minn(Src0, C0) - maxx(Src1, C1),
        reference=lambda in0, in1, s0, s1, imm2: (
            np.minimum(in0.astype(np.float32), s0) - np.maximum(in1, s1)
        ).astype(np.float32),
    )
    shas = {}
    for ver in ("v3", "v4"):
        try:
            uops = lower(spec, ver=ver)
            shas[ver] = DveOpSpec(
                name="WSUB_ANT", opcode=0, uops=uops, rd1_en=True
            ).sha(ver)
        except Exception:
            pass
    op = dve_ops.DveOp("WSUB_ANT", spec, subdim=False, uops_sha=shas)
    dve_ops.OPS.append(op)
    dve_ops.CUSTOM_DVE_SPECS[op.name] = spec
    dve_ops._SUB_OPCODE_FOR_NAME[op.name] = (
        max(dve_ops._SUB_OPCODE_FOR_NAME.values()) + 1
    )
    assert dve_ops._SUB_OPCODE_FOR_NAME[op.name] < 0x20
    return op


def _build_kernel(reps=1):
    try:
        wsub = _register_wsub()
    except Exception:
        wsub = None
    nc = bacc.Bacc("TRN2", target_bir_lowering=False, debug=False,
                   num_devices=NCORES)

    pl = nc.dram_tensor("pred_logits", [BPC, Q, 1], F32, kind="ExternalInput").ap()
    pb = nc.dram_tensor("pred_boxes", [BPC, Q, 4], F32, kind="ExternalInput").ap()
    tb = nc.dram_tensor("tgt_boxes", [BPC, G, 4], F32, kind="ExternalInput").ap()

    vals_o = nc.dram_tensor("vals", [BPC, G, K], F32, kind="ExternalOutput").ap()
    idxs_o = nc.dram_tensor("idxs", [BPC, G, K], I32, kind="ExternalOutput").ap()
    mask_o = nc.dram_tensor("mask", [BPC, G, K], U8, kind="ExternalOutput").ap()

    QP = Q // 8  # 125 queries per partition in packed layout

    with tile.TileContext(nc) as tc, ExitStack() as ctx:
        dram = ctx.enter_context(tc.tile_pool(name="dram", bufs=1, space="DRAM"))
        const = ctx.enter_context(tc.tile_pool(name="const", bufs=1))
        prep = ctx.enter_context(tc.tile_pool(name="prep", bufs=1))
        rows = ctx.enter_context(tc.tile_pool(name="rows", bufs=2))
        lines = ctx.enter_context(tc.tile_pool(name="lines", bufs=2))
        work = ctx.enter_context(tc.tile_pool(name="work", bufs=2))
        outp = ctx.enter_context(tc.tile_pool(name="outp", bufs=1))
        psum = ctx.enter_context(tc.tile_pool(name="psum", bufs=4, space="PSUM"))

        # ---- phase 0a: packed per-query values -> DRAM lines -------------
        # linepack layout: [6, BPC, Q] rows = px1,py1,px2,py2,pa,-score so a
        # single [6,Q] DMA (6 partitions, contiguous 4KB runs) feeds all 6
        # broadcasts of one image.
        lp_d = dram.tile([6 * BPC * Q], F32, tag="linepack")
        lpv = lp_d[:].rearrange("(j b q) -> j b q", b=BPC, q=Q)

        def pack_row(j, tile_view):
            # tile_view: [128, QP] SBUF, partition 8b+ph = queries of image b
            nc.sync.dma_start(
                lpv[j, :, :].rearrange("b (ph r) -> (b ph) r", ph=8),
                tile_view,
            )

        # packed pred boxes: partition p holds queries [QP*p, QP*p+QP)
        pbt = prep.tile([128, QP * 4], F32, tag="pbt")
        nc.sync.dma_start(
            pbt[:],
            pb.rearrange("b q c -> (b q c)").rearrange("(p x) -> p x", p=128),
        )
        pv = pbt[:].rearrange("p (r c) -> p r c", c=4)
        # deinterleave coords so pack DMAs are contiguous (125 x 500B runs
        # instead of 16000 x 4B descriptors)
        for c in range(4):
            pc = prep.tile([128, QP], F32, tag=f"pc{c}")
            nc.vector.tensor_copy(pc[:], pv[:, :, c])
            pack_row(c, pc[:])
        dx = prep.tile([128, QP], F32, tag="dx")
        dy = prep.tile([128, QP], F32, tag="dy")
        pa = prep.tile([128, QP], F32, tag="pa")
        nc.vector.tensor_tensor(dx[:], pv[:, :, 2], pv[:, :, 0], Op.subtract)
        nc.vector.tensor_tensor(dy[:], pv[:, :, 3], pv[:, :, 1], Op.subtract)
        nc.vector.tensor_tensor(pa[:], dx[:], dy[:], Op.mult)
        pack_row(4, pa[:])

        # -score = -sigmoid(logit) = recip(-(1 + exp(-x))), 2-ULP recip
        lg = prep.tile([128, QP], F32, tag="lg")
        nc.sync.dma_start(
            lg[:],
            pl.rearrange("b q c -> (b q c)").rearrange("(p x) -> p x", p=128),
        )
        ex = prep.tile([128, QP], F32, tag="ex")
        nc.scalar.activation(ex[:], lg[:], mybir.ActivationFunctionType.Exp,
                             scale=-1.0)
        # nw = -(1 + exp(-x)); nsc = 1/nw = -sigmoid(x) at ~2 ULP
        nw = prep.tile([128, QP], F32, tag="nw")
        nc.vector.tensor_scalar(nw[:], ex[:], 1.0, -1.0, Op.add, Op.mult)
        nsc = prep.tile([128, QP], F32, tag="nsc")
        nscr = prep.tile([128, QP], F32, tag="nscr")
        nc.vector.reciprocal_approx_accurate(nsc[:], nw[:], nscr[:])
        pack_row(5, nsc[:])

        # ---- phase 0b: per-target scalars for all images -----------------
        # tsc_all[p, ci*64 + b*4 + c] = tgt coord c of g=ci*128+p, image b
        tsc_all = const.tile([128, 3 * BPC * 4], F32, tag="tsc_all")
        nc.vector.memset(tsc_all[:], 0.0)  # partitions >=44 of ci=2 unused
        for ci in range(3):
            P = min(128, G - ci * 128)
            nc.sync.dma_start(
                tsc_all[0:P, ci * 64:(ci + 1) * 64].rearrange(
                    "p (b c) -> p b c", c=4),
                tb[:, ci * 128:ci * 128 + P, :].rearrange("b p c -> p b c"),
            )
        tv = tsc_all[:].rearrange("p (i c) -> p i c", c=4)  # i = ci*16+b
        tdx = const.tile([128, 48], F32, tag="tdx")
        tdy = const.tile([128, 48], F32, tag="tdy")
        tar = const.tile([128, 48], F32, tag="tar")
        ta_all = const.tile([128, 48], F32, tag="ta_all")
        nc.vector.tensor_tensor(tdx[:], tv[:, :, 2], tv[:, :, 0], Op.subtract)
        nc.vector.tensor_tensor(tdy[:], tv[:, :, 3], tv[:, :, 1], Op.subtract)
        nc.vector.tensor_tensor(tar[:], tdx[:], tdy[:], Op.mult)
        # fold the union epsilon into the target area: ta' = ta + 1e-7
        nc.vector.tensor_scalar(ta_all[:], tar[:], 1e-7, None, Op.add)

        # ones row for PE-based partition broadcast
        ones = const.tile([1, 128], F32, tag="ones")
        nc.vector.memset(ones[:], 1.0)

        # tie-break bias row: (Q - q) * 2^-40, identical on all partitions
        bias_i = const.tile([128, Q], I32, tag="bias_i")
        nc.gpsimd.iota(bias_i[:], pattern=[[-1, Q]], base=Q,
                       channel_multiplier=0)
        bias_f = const.tile([128, Q], F32, tag="bias_f")
        nc.vector.tensor_scalar(bias_f[:], bias_i[:], BIAS_SCALE, None, Op.mult)

        # ---- output collectors (all images) ------------------------------
        v8s = outp.tile([128, BPC * 24], F32, tag="v8s")  # top8 per (b,ci)
        i8s = outp.tile([128, BPC * 24], U32, tag="i8s")
        vall = outp.tile([128, BPC * 12], F32, tag="vall")
        mall = outp.tile([128, BPC * 12], U8, tag="mall")
        nc.gpsimd.memset(v8s[:], 0.0)

        # ---- main loop: per image, per g-chunk ---------------------------
        for b in [bb for _ in range(reps) for bb in range(BPC)]:
            r_px1 = rows.tile([128, Q], F32, tag="px1")
            r_py1 = rows.tile([128, Q], F32, tag="py1")
            r_px2 = rows.tile([128, Q], F32, tag="px2")
            r_py2 = rows.tile([128, Q], F32, tag="py2")
            r_pa = rows.tile([128, Q], F32, tag="pa")
            r_nsc = rows.tile([128, Q], F32, tag="nsc")
            all_rows = (r_px1, r_py1, r_px2, r_py2, r_pa, r_nsc)

            # one [1,6Q] line DMA per image (6 contiguous 4KB runs), then PE
            # ones-matmul broadcast (bit-exact 1.0*x) + ScalarE copies
            ln = lines.tile([1, 6 * Q], F32, tag="line")
            nc.sync.dma_start(
                ln[:].rearrange("a (j q) -> a j q", j=6),
                lpv[:, b, :].rearrange("(a j) q -> a j q", a=1),
            )
            HB = 500  # psum bank-sized matmul piece (N<=512)
            for j, rt in enumerate(all_rows):
                pt = psum.tile([128, 1024], F32, tag="pt")
                for h in range(Q // HB):
                    nc.tensor.matmul(pt[:, h * 512:h * 512 + HB],
                                     ones[:],
                                     ln[0:1, j * Q + h * HB:j * Q + (h + 1) * HB],
                                     start=True, stop=True)
                nc.scalar.activation(
                    rt[:].rearrange("p (h x) -> p h x", h=2),
                    pt[:].rearrange("p (h x) -> p h x", h=2)[:, :, 0:HB],
                    mybir.ActivationFunctionType.Copy)

            for ci in range(3):
                P = min(128, G - ci * 128)
                i0 = ci * 16 + b  # column index into tsc_all/ta_all
                tx1 = tv[0:P, i0, 0:1]
                ty1 = tv[0:P, i0, 1:2]
                tx2 = tv[0:P, i0, 2:3]
                ty2 = tv[0:P, i0, 3:4]
                ta = ta_all[0:P, i0:i0 + 1]

                def wsub_op(out_t, hi_row, lo_row, hi_s, lo_s):
                    if wsub is not None:
                        nc.vector._custom_dve(wsub, out=out_t[:], in0=hi_row,
                                              in1=lo_row, s0=hi_s, s1=lo_s)
                    else:
                        lt = work.tile([P, Q], F32, tag="H")
                        nc.vector.tensor_scalar(lt[:], lo_row, lo_s, None,
                                                Op.max)
                        nc.vector.scalar_tensor_tensor(out_t[:], hi_row, hi_s,
                                                       lt[:], Op.min,
                                                       Op.subtract)

                ACT = mybir.ActivationFunctionType
                # DVE: overlap widths, one fused custom op each
                wx = work.tile([P, Q], F32, tag="A")
                wsub_op(wx, r_px2[:P], r_px1[:P], tx2, tx1)
                wy = work.tile([P, Q], F32, tag="B")
                wsub_op(wy, r_py2[:P], r_py1[:P], ty2, ty1)
                # Act+Pool: inter = relu(wx) * wy
                rwx = work.tile([P, Q], F32, tag="C")
                nc.scalar.activation(rwx[:], wx[:], ACT.Relu)
                inter = work.tile([P, Q], F32, tag="A")
                nc.gpsimd.tensor_tensor(inter[:], rwx[:], wy[:], Op.mult)
                # Pool+Act: Up = (pa - inter) + ta'   (eps already in ta')
                U0 = work.tile([P, Q], F32, tag="B")
                nc.gpsimd.tensor_tensor(U0[:], r_pa[:P], inter[:], Op.subtract)
                Up = work.tile([P, Q], F32, tag="D")
                nc.scalar.activation(Up[:], U0[:], ACT.Identity, bias=ta)
                # DVE: fast recip seed (~51 ULP)
                r0 = work.tile([P, Q], F32, tag="E")
                nc.vector.reciprocal_approx_fast(r0[:], Up[:])
                # Pool+Act+Pool: NR correction, negated:
                # negr = (Up*r0 - 2) * r0 = -R
                t_ = work.tile([P, Q], F32, tag="F")
                nc.gpsimd.tensor_tensor(t_[:], Up[:], r0[:], Op.mult)
                tm2 = work.tile([P, Q], F32, tag="G")
                nc.scalar.activation(tm2[:], t_[:], ACT.Copy, bias=-2.0)
                negr = work.tile([P, Q], F32, tag="F")
                nc.gpsimd.tensor_tensor(negr[:], tm2[:], r0[:], Op.mult)
                # Pool: q1 = (inter * -R) * (-score) = score * iou
                p1 = work.tile([P, Q], F32, tag="C")
                nc.gpsimd.tensor_tensor(p1[:], inter[:], negr[:], Op.mult)
                q1 = work.tile([P, Q], F32, tag="G")
                nc.gpsimd.tensor_tensor(q1[:], p1[:], r_nsc[:P], Op.mult)
                # Act+Pool: negd = 0.4*Up - inter  (valid <=> negd < 0)
                u04 = work.tile([P, Q], F32, tag="B")
                nc.scalar.activation(u04[:], Up[:], ACT.Copy, scale=0.4)
                negd = work.tile([P, Q], F32, tag="D")
                nc.gpsimd.tensor_tensor(negd[:], u04[:], inter[:], Op.subtract)
                # DVE: sel = (negd < 0) * q1
                sel = work.tile([P, Q], F32, tag="E")
                nc.vector.scalar_tensor_tensor(sel[:], negd[:], 0.0, q1[:],
                                               Op.is_lt, Op.mult)
                # Pool: m3 = sel + bias
                m3 = work.tile([P, Q], F32, tag="F")
                nc.gpsimd.tensor_tensor(m3[:], sel[:], bias_f[:P], Op.add)

                c0 = b * 24 + ci * 8
                v8 = v8s[0:P, c0:c0 + 8]
                nc.vector.max(v8, m3[:])
                nc.vector.max_index(i8s[0:P, c0:c0 + 8], v8, m3[:])

            # batched epilogue once per rep at the last image
            if b == BPC - 1:
                v8v = v8s[:].rearrange("p (b c e) -> p b c e", b=BPC, e=8)[
                    :, :, :, 0:K]
                nc.vector.scalar_tensor_tensor(
                    vall[:].rearrange("p (b c e) -> p b c e", b=BPC, e=K),
                    v8v, POS_THRESH, v8v, Op.is_gt, Op.mult)
                nc.vector.tensor_scalar(
                    mall[:].rearrange("p (b c e) -> p b c e", b=BPC, e=K),
                    v8v, POS_THRESH, None, Op.is_gt)

                # one DMA per (output, g-chunk): 9 total, [P,16,4] each
                i8v = i8s[:].rearrange("p (b c e) -> p b c e", b=BPC, e=8)
                vav = vall[:].rearrange("p (b c e) -> p b c e", b=BPC, e=K)
                mav = mall[:].rearrange("p (b c e) -> p b c e", b=BPC, e=K)
                for ci in range(3):
                    P = min(128, G - ci * 128)
                    dst = slice(ci * 128, ci * 128 + P)
                    nc.sync.dma_start(
                        vals_o[:, dst, :].rearrange("b p k -> p b k"),
                        vav[0:P, :, ci, :])
                    nc.scalar.dma_start(
                        idxs_o[:, dst, :].rearrange("b p k -> p b k"),
                        i8v[0:P, :, ci, 0:K].bitcast(I32))
                    nc.gpsimd.dma_start(
                        mask_o[:, dst, :].rearrange("b p k -> p b k"),
                        mav[0:P, :, ci, :])

    nc.compile()
    return nc


_NC = None


def _get_nc():
    global _NC
    if _NC is None:
        _NC = _build_kernel()
    return _NC


def run(pred_logits, pred_boxes_xyxy, tgt_boxes_xyxy, **spmd_kwargs):
    nc = _get_nc()
    pred_logits = np.ascontiguousarray(np.asarray(pred_logits, dtype=np.float32))
    pred_boxes = np.ascontiguousarray(np.asarray(pred_boxes_xyxy, dtype=np.float32))
    tgt_boxes = np.ascontiguousarray(np.asarray(tgt_boxes_xyxy, dtype=np.float32))
    in_maps = [
        {
            "pred_logits": pred_logits[c * BPC:(c + 1) * BPC],
            "pred_boxes": pred_boxes[c * BPC:(c + 1) * BPC],
            "tgt_boxes": tgt_boxes[c * BPC:(c + 1) * BPC],
        }
        for c in range(NCORES)
    ]
    res = run_bass_kernel_spmd(nc, in_maps, list(range(NCORES)), **spmd_kwargs)
    vals = np.concatenate([res.results[c]["vals"] for c in range(NCORES)], axis=0)
    idxs = np.concatenate([res.results[c]["idxs"] for c in range(NCORES)], axis=0)
    mask = np.concatenate([res.results[c]["mask"] for c in range(NCORES)], axis=0)
    return (vals, idxs.astype(np.int32), mask.astype(bool)), res


def kernel(pred_logits, pred_boxes_xyxy, tgt_boxes_xyxy):
    (vals, idxs, mask), _ = run(pred_logits, pred_boxes_xyxy, tgt_boxes_xyxy)
    return vals, idxs, mask


# revision 20
# speedup vs baseline: 1.0910x; 1.0910x over previous
"""BinaryOneToManyMatcher (nms_detection) Trainium2 Bass kernel.

Computes, for B=128 images with Q=1000 predicted boxes and G=300 GT boxes:
  score = sigmoid(pred_logits)            [B,Q]
  iou   = pairwise IoU(pred, tgt)         [B,Q,G]
  gt    = score * iou * (iou > 0.4)       [B,Q,G]
  vals, idxs = top_k(gt over Q, k=4); mask = vals > 0

Sharding: pure data parallel, 16 images per NeuronCore across 8 cores.

Per-core layout: per image, G on partitions (chunks of <=128) and Q on the
free dim.  Per-query rows (x1,y1,x2,y2,area,-score) are packed to DRAM once,
loaded per image as one contiguous [1,6Q] line and broadcast across
partitions via PE ones-matmul (bit-exact).  Per-target values are [P,1]
per-partition scalars loaded in 3 batched DMAs for all 16 images.

Work split per g-chunk (balanced DVE/Pool):
  DVE : wx, wy (fused min-sub-max customs), recip seed, mask-select,
        Max8, MaxIndex
  Pool: inter, Up, NR correction (2 ops), iou*score (2 ops), negd, +bias
  Act : 6 PSUM->SBUF broadcast-row copies per image
  PE  : 12 broadcast matmuls per image

Exactness: idxs/mask are bit-exact vs the reference on the dataset.  The
valid decision uses negd = 0.4*Up - inter (sign test, no division); the
per-q tie-break bias of 2^-40 makes every row value distinct so Max8 ties
resolve to ascending q, matching jax.lax.top_k.  The NR-corrected
reciprocal (~2 ULP) only affects vals (loose tol) and top-4 ordering
(dataset margin 4.7e-6 >> 2 ULP).
"""

import os
from contextlib import ExitStack

import numpy as np

import concourse.bass as bass
import concourse.tile as tile
from concourse import bacc, mybir
from concourse.bass_utils import run_bass_kernel_spmd

B, Q, G, K = 128, 1000, 300, 4
NCORES = 8
BPC = B // NCORES  # images per core

F32 = mybir.dt.float32
I32 = mybir.dt.int32
U32 = mybir.dt.uint32
U8 = mybir.dt.uint8
Op = mybir.AluOpType

BIAS_SCALE = float(2.0**-40)  # per-q tie-break bias scale
POS_THRESH = 1e-6  # separates real positives (>=3e-3) from bias (<1e-9)


def _register_wsub():
    """Custom DVE op: out = min(in0, s0) - max(in1, s1) in one pass."""
    from concourse import dve_ops
    from concourse.dve_spec import Spec, Src0, Src1, C0, C1, minn, maxx, lower
    from concourse.dve_uop import DveOpSpec

    for op in dve_ops.OPS:
        if op.name == "WSUB_ANT":
            return op

    spec = Spec(
        body=minn(Src0, C0) - maxx(Src1, C1),
        reference=lambda in0, in1, s0, s1, imm2: (
            np.minimum(in0.astype(np.float32), s0) - np.maximum(in1, s1)
        ).astype(np.float32),
    )
    shas = {}
    for ver in ("v3", "v4"):
        try:
            uops = lower(spec, ver=ver)
            shas[ver] = DveOpSpec(
                name="WSUB_ANT", opcode=0, uops=uops, rd1_en=True
            ).sha(ver)
        except Exception:
            pass
    op = dve_ops.DveOp("WSUB_ANT", spec, subdim=False, uops_sha=shas)
    dve_ops.OPS.append(op)
    dve_ops.CUSTOM_DVE_SPECS[op.name] = spec
    dve_ops._SUB_OPCODE_FOR_NAME[op.name] = (
        max(dve_ops._SUB_OPCODE_FOR_NAME.values()) + 1
    )
    assert dve_ops._SUB_OPCODE_FOR_NAME[op.name] < 0x20
    return op


def _build_kernel(reps=1):
    try:
        wsub = _register_wsub()
    except Exception:
        wsub = None
    nc = bacc.Bacc("TRN2", target_bir_lowering=False, debug=False,
                   num_devices=NCORES)

    pl = nc.dram_tensor("pred_logits", [BPC, Q, 1], F32, kind="ExternalInput").ap()
    pb = nc.dram_tensor("pred_boxes", [BPC, Q, 4], F32, kind="ExternalInput").ap()
    tb = nc.dram_tensor("tgt_boxes", [BPC, G, 4], F32, kind="ExternalInput").ap()

    vals_o = nc.dram_tensor("vals", [BPC, G, K], F32, kind="ExternalOutput").ap()
    idxs_o = nc.dram_tensor("idxs", [BPC, G, K], I32, kind="ExternalOutput").ap()
    mask_o = nc.dram_tensor("mask", [BPC, G, K], U8, kind="ExternalOutput").ap()

    QP = Q // 8  # 125 queries per partition in packed layout

    with tile.TileContext(nc) as tc, ExitStack() as ctx:
        dram = ctx.enter_context(tc.tile_pool(name="dram", bufs=1, space="DRAM"))
        const = ctx.enter_context(tc.tile_pool(name="const", bufs=1))
        prep = ctx.enter_context(tc.tile_pool(name="prep", bufs=1))
        rows = ctx.enter_context(tc.tile_pool(name="rows", bufs=2))
        lines = ctx.enter_context(tc.tile_pool(name="lines", bufs=2))
        work = ctx.enter_context(tc.tile_pool(name="work", bufs=2))
        outp = ctx.enter_context(tc.tile_pool(name="outp", bufs=1))
        psum = ctx.enter_context(tc.tile_pool(name="psum", bufs=4, space="PSUM"))

        # ---- phase 0a: packed per-query values -> DRAM lines -------------
        # linepack layout: [6, BPC, Q] rows = px1,py1,px2,py2,pa,-score so a
        # single [6,Q] DMA (6 partitions, contiguous 4KB runs) feeds all 6
        # broadcasts of one image.
        lp_d = dram.tile([6 * BPC * Q], F32, tag="linepack")
        lpv = lp_d[:].rearrange("(j b q) -> j b q", b=BPC, q=Q)

        def pack_row(j, tile_view):
            # tile_view: [128, QP] SBUF, partition 8b+ph = queries of image b
            nc.sync.dma_start(
                lpv[j, :, :].rearrange("b (ph r) -> (b ph) r", ph=8),
                tile_view,
            )

        # packed pred boxes: partition p holds queries [QP*p, QP*p+QP)
        pbt = prep.tile([128, QP * 4], F32, tag="pbt")
        nc.sync.dma_start(
            pbt[:],
            pb.rearrange("b q c -> (b q c)").rearrange("(p x) -> p x", p=128),
        )
        pv = pbt[:].rearrange("p (r c) -> p r c", c=4)
        # deinterleave coords so pack DMAs are contiguous (125 x 500B runs
        # instead of 16000 x 4B descriptors)
        for c in range(4):
            pc = prep.tile([128, QP], F32, tag=f"pc{c}")
            nc.vector.tensor_copy(pc[:], pv[:, :, c])
            pack_row(c, pc[:])
        dx = prep.tile([128, QP], F32, tag="dx")
        dy = prep.tile([128, QP], F32, tag="dy")
        pa = prep.tile([128, QP], F32, tag="pa")
        nc.vector.tensor_tensor(dx[:], pv[:, :, 2], pv[:, :, 0], Op.subtract)
        nc.vector.tensor_tensor(dy[:], pv[:, :, 3], pv[:, :, 1], Op.subtract)
        nc.vector.tensor_tensor(pa[:], dx[:], dy[:], Op.mult)
        pack_row(4, pa[:])

        # -score = -sigmoid(logit) = recip(-(1 + exp(-x))), 2-ULP recip
        lg = prep.tile([128, QP], F32, tag="lg")
        nc.sync.dma_start(
            lg[:],
            pl.rearrange("b q c -> (b q c)").rearrange("(p x) -> p x", p=128),
        )
        ex = prep.tile([128, QP], F32, tag="ex")
        nc.scalar.activation(ex[:], lg[:], mybir.ActivationFunctionType.Exp,
                             scale=-1.0)
        # nw = -(1 + exp(-x)); nsc = 1/nw = -sigmoid(x) at ~2 ULP
        nw = prep.tile([128, QP], F32, tag="nw")
        nc.vector.tensor_scalar(nw[:], ex[:], 1.0, -1.0, Op.add, Op.mult)
        nsc = prep.tile([128, QP], F32, tag="nsc")
        nscr = prep.tile([128, QP], F32, tag="nscr")
        nc.vector.reciprocal_approx_accurate(nsc[:], nw[:], nscr[:])
        pack_row(5, nsc[:])

        # ---- phase 0b: per-target scalars for all images -----------------
        # tsc_all[p, ci*64 + b*4 + c] = tgt coord c of g=ci*128+p, image b
        tsc_all = const.tile([128, 3 * BPC * 4], F32, tag="tsc_all")
        nc.vector.memset(tsc_all[:], 0.0)  # partitions >=44 of ci=2 unused
        for ci in range(3):
            P = min(128, G - ci * 128)
            nc.sync.dma_start(
                tsc_all[0:P, ci * 64:(ci + 1) * 64].rearrange(
                    "p (b c) -> p b c", c=4),
                tb[:, ci * 128:ci * 128 + P, :].rearrange("b p c -> p b c"),
            )
        tv = tsc_all[:].rearrange("p (i c) -> p i c", c=4)  # i = ci*16+b
        tdx = const.tile([128, 48], F32, tag="tdx")
        tdy = const.tile([128, 48], F32, tag="tdy")
        tar = const.tile([128, 48], F32, tag="tar")
        ta_all = const.tile([128, 48], F32, tag="ta_all")
        nc.vector.tensor_tensor(tdx[:], tv[:, :, 2], tv[:, :, 0], Op.subtract)
        nc.vector.tensor_tensor(tdy[:], tv[:, :, 3], tv[:, :, 1], Op.subtract)
        nc.vector.tensor_tensor(tar[:], tdx[:], tdy[:], Op.mult)
        # fold the union epsilon into the target area: ta' = ta + 1e-7
        nc.vector.tensor_scalar(ta_all[:], tar[:], 1e-7, None, Op.add)

        # ones row for PE-based partition broadcast
        ones = const.tile([1, 128], F32, tag="ones")
        nc.vector.memset(ones[:], 1.0)

        # tie-break bias row: (Q - q) * 2^-40, identical on all partitions
        bias_i = const.tile([128, Q], I32, tag="bias_i")
        nc.gpsimd.iota(bias_i[:], pattern=[[-1, Q]], base=Q,
                       channel_multiplier=0)
        bias_f = const.tile([128, Q], F32, tag="bias_f")
        nc.vector.tensor_scalar(bias_f[:], bias_i[:], BIAS_SCALE, None, Op.mult)

        # ---- output collectors (all images) ------------------------------
        v8s = outp.tile([128, BPC * 24], F32, tag="v8s")  # top8 per (b,ci)
        i8s = outp.tile([128, BPC * 24], U32, tag="i8s")
        vall = outp.tile([128, BPC * 12], F32, tag="vall")
        mall = outp.tile([128, BPC * 12], U8, tag="mall")
        nc.gpsimd.memset(v8s[:], 0.0)

        # ---- main loop: per image, per g-chunk ---------------------------
        # pending topk work is emitted one chunk late (software pipelining):
        # Max8/MaxIndex of chunk c enter the DVE stream after chunk c+1's
        # wx/wy, so the DVE fills the wait for chunk c+1's Pool/Act chain
        # with useful work instead of head-of-line blocking.
        pending = []

        def flush_topk():
            for v8ap, i8ap, m3ap in pending:
                nc.vector.max(v8ap, m3ap)
                nc.vector.max_index(i8ap, v8ap, m3ap)
            pending.clear()

        for b in [bb for _ in range(reps) for bb in range(BPC)]:
            r_px1 = rows.tile([128, Q], F32, tag="px1")
            r_py1 = rows.tile([128, Q], F32, tag="py1")
            r_px2 = rows.tile([128, Q], F32, tag="px2")
            r_py2 = rows.tile([128, Q], F32, tag="py2")
            r_pa = rows.tile([128, Q], F32, tag="pa")
            r_nsc = rows.tile([128, Q], F32, tag="nsc")
            all_rows = (r_px1, r_py1, r_px2, r_py2, r_pa, r_nsc)

            # one [1,6Q] line DMA per image (6 contiguous 4KB runs), then PE
            # ones-matmul broadcast (bit-exact 1.0*x) + ScalarE copies
            ln = lines.tile([1, 6 * Q], F32, tag="line")
            nc.sync.dma_start(
                ln[:].rearrange("a (j q) -> a j q", j=6),
                lpv[:, b, :].rearrange("(a j) q -> a j q", a=1),
            )
            HB = 500  # psum bank-sized matmul piece (N<=512)
            for j, rt in enumerate(all_rows):
                pt = psum.tile([128, 1024], F32, tag="pt")
                for h in range(Q // HB):
                    nc.tensor.matmul(pt[:, h * 512:h * 512 + HB],
                                     ones[:],
                                     ln[0:1, j * Q + h * HB:j * Q + (h + 1) * HB],
                                     start=True, stop=True)
                nc.scalar.activation(
                    rt[:].rearrange("p (h x) -> p h x", h=2),
                    pt[:].rearrange("p (h x) -> p h x", h=2)[:, :, 0:HB],
                    mybir.ActivationFunctionType.Copy)

            for ci in range(3):
                P = min(128, G - ci * 128)
                i0 = ci * 16 + b  # column index into tsc_all/ta_all
                tx1 = tv[0:P, i0, 0:1]
                ty1 = tv[0:P, i0, 1:2]
                tx2 = tv[0:P, i0, 2:3]
                ty2 = tv[0:P, i0, 3:4]
                ta = ta_all[0:P, i0:i0 + 1]

                def wsub_op(out_t, hi_row, lo_row, hi_s, lo_s):
                    if wsub is not None:
                        nc.vector._custom_dve(wsub, out=out_t[:], in0=hi_row,
                                              in1=lo_row, s0=hi_s, s1=lo_s)
                    else:
                        lt = work.tile([P, Q], F32, tag="H")
                        nc.vector.tensor_scalar(lt[:], lo_row, lo_s, None,
                                                Op.max)
                        nc.vector.scalar_tensor_tensor(out_t[:], hi_row, hi_s,
                                                       lt[:], Op.min,
                                                       Op.subtract)

                ACT = mybir.ActivationFunctionType
                # DVE: overlap widths, one fused custom op each
                wx = work.tile([P, Q], F32, tag="A")
                wsub_op(wx, r_px2[:P], r_px1[:P], tx2, tx1)
                wy = work.tile([P, Q], F32, tag="B")
                wsub_op(wy, r_py2[:P], r_py1[:P], ty2, ty1)
                flush_topk()  # previous chunk's Max8/MaxIndex
                # Act+Pool: inter = relu(wx) * wy
                rwx = work.tile([P, Q], F32, tag="C")
                nc.scalar.activation(rwx[:], wx[:], ACT.Relu)
                inter = work.tile([P, Q], F32, tag="I")
                nc.gpsimd.tensor_tensor(inter[:], rwx[:], wy[:], Op.mult)
                # Pool+Act: Up = (pa - inter) + ta'   (eps already in ta')
                U0 = work.tile([P, Q], F32, tag="B")
                nc.gpsimd.tensor_tensor(U0[:], r_pa[:P], inter[:], Op.subtract)
                Up = work.tile([P, Q], F32, tag="D")
                nc.scalar.activation(Up[:], U0[:], ACT.Identity, bias=ta)
                # DVE: fast recip seed (~51 ULP)
                r0 = work.tile([P, Q], F32, tag="E")
                nc.vector.reciprocal_approx_fast(r0[:], Up[:])
                # Pool+Act+Pool: NR correction, negated:
                # negr = (Up*r0 - 2) * r0 = -R
                t_ = work.tile([P, Q], F32, tag="F")
                nc.gpsimd.tensor_tensor(t_[:], Up[:], r0[:], Op.mult)
                tm2 = work.tile([P, Q], F32, tag="G")
                nc.scalar.activation(tm2[:], t_[:], ACT.Copy, bias=-2.0)
                negr = work.tile([P, Q], F32, tag="F")
                nc.gpsimd.tensor_tensor(negr[:], tm2[:], r0[:], Op.mult)
                # Pool: q1 = (inter * -R) * (-score) = score * iou
                p1 = work.tile([P, Q], F32, tag="C")
                nc.gpsimd.tensor_tensor(p1[:], inter[:], negr[:], Op.mult)
                q1 = work.tile([P, Q], F32, tag="G")
                nc.gpsimd.tensor_tensor(q1[:], p1[:], r_nsc[:P], Op.mult)
                # Act+Pool: negd = 0.4*Up - inter  (valid <=> negd < 0)
                u04 = work.tile([P, Q], F32, tag="B")
                nc.scalar.activation(u04[:], Up[:], ACT.Copy, scale=0.4)
                negd = work.tile([P, Q], F32, tag="D")
                nc.gpsimd.tensor_tensor(negd[:], u04[:], inter[:], Op.subtract)
                # DVE: sel = (negd < 0) * q1
                sel = work.tile([P, Q], F32, tag="E")
                nc.vector.scalar_tensor_tensor(sel[:], negd[:], 0.0, q1[:],
                                               Op.is_lt, Op.mult)
                # Pool: m3 = sel + bias
                m3 = work.tile([P, Q], F32, tag="M")
                nc.gpsimd.tensor_tensor(m3[:], sel[:], bias_f[:P], Op.add)

                c0 = b * 24 + ci * 8
                pending.append(
                    (v8s[0:P, c0:c0 + 8], i8s[0:P, c0:c0 + 8], m3[:]))

            # batched epilogue once per rep at the last image
            if b == BPC - 1:
                flush_topk()
                v8v = v8s[:].rearrange("p (b c e) -> p b c e", b=BPC, e=8)[
                    :, :, :, 0:K]
                nc.vector.scalar_tensor_tensor(
                    vall[:].rearrange("p (b c e) -> p b c e", b=BPC, e=K),
                    v8v, POS_THRESH, v8v, Op.is_gt, Op.mult)
                nc.vector.tensor_scalar(
                    mall[:].rearrange("p (b c e) -> p b c e", b=BPC, e=K),
                    v8v, POS_THRESH, None, Op.is_gt)

                # one DMA per (output, g-chunk): 9 total, [P,16,4] each
                i8v = i8s[:].rearrange("p (b c e) -> p b c e", b=BPC, e=8)
                vav = vall[:].rearrange("p (b c e) -> p b c e", b=BPC, e=K)
                mav = mall[:].rearrange("p (b c e) -> p b c e", b=BPC, e=K)
                for ci in range(3):
                    P = min(128, G - ci * 128)
                    dst = slice(ci * 128, ci * 128 + P)
                    nc.sync.dma_start(
                        vals_o[:, dst, :].rearrange("b p k -> p b k"),
                        vav[0:P, :, ci, :])
                    nc.scalar.dma_start(
                        idxs_o[:, dst, :].rearrange("b p k -> p b k"),
                        i8v[0:P, :, ci, 0:K].bitcast(I32))
                    nc.gpsimd.dma_start(
                        mask_o[:, dst, :].rearrange("b p k -> p b k"),
                        mav[0:P, :, ci, :])

    nc.compile()
    return nc


_NC = None


def _get_nc():
    global _NC
    if _NC is None:
        _NC = _build_kernel()
    return _NC


def run(pred_logits, pred_boxes_xyxy, tgt_boxes_xyxy, **spmd_kwargs):
    nc = _get_nc()
    pred_logits = np.ascontiguousarray(np.asarray(pred_logits, dtype=np.float32))
    pred_boxes = np.ascontiguousarray(np.asarray(pred_boxes_xyxy, dtype=np.float32))
    tgt_boxes = np.ascontiguousarray(np.asarray(tgt_boxes_xyxy, dtype=np.float32))
    in_maps = [
        {
            "pred_logits": pred_logits[c * BPC:(c + 1) * BPC],
            "pred_boxes": pred_boxes[c * BPC:(c + 1) * BPC],
            "tgt_boxes": tgt_boxes[c * BPC:(c + 1) * BPC],
        }
        for c in range(NCORES)
    ]
    res = run_bass_kernel_spmd(nc, in_maps, list(range(NCORES)), **spmd_kwargs)
    vals = np.concatenate([res.results[c]["vals"] for c in range(NCORES)], axis=0)
    idxs = np.concatenate([res.results[c]["idxs"] for c in range(NCORES)], axis=0)
    mask = np.concatenate([res.results[c]["mask"] for c in range(NCORES)], axis=0)
    return (vals, idxs.astype(np.int32), mask.astype(bool)), res


def kernel(pred_logits, pred_boxes_xyxy, tgt_boxes_xyxy):
    (vals, idxs, mask), _ = run(pred_logits, pred_boxes_xyxy, tgt_boxes_xyxy)
    return vals, idxs, mask
